# revision 7
# baseline (speedup 1.0000x reference)
"""Gated max/avg 2x2 pooling for Trainium2 — custom-DVE pair-rate kernel.

Reference computation (per 2x2 window over [B, H, W, C], stride 2):
    x1 = max(window), x2 = mean(window)
    xs = sum_ij mask[i, j] * window[i, j]   (per channel)
    out = sigmoid(xs) * x1 + (1 - sigmoid(xs)) * x2

Sharding: pure data-parallel over batch (16 batches -> 2 per core).

v3 design: three hand-authored custom DVE ops run in the 2X_1PORT perf
mode, where each SBUF read port delivers a packed fp16 pair per cycle.
The host packs x so port0 streams the window's top-row pair (a,b) and
port1 the bottom-row pair (c,d); each op then consumes all four window
values per cycle through the SRC_0/SRC_0_HI/SRC_1/SRC_1_HI crossbar
lanes and folds the whole reduction into the 8-stage ALU pipeline:

  PAIRPOOL: [x1 | x2] pairs  (max tree + sum tree + 0.25 scale, 8 ALUs)
  PAIRDOT:  [t3 | t3] pairs  (t3 = va*a + vb*b + vc*c + d = xs/f)
  GATE:     out = x2 + z*(x1 - x2)   (reads [x1|x2] pairs + z pairs)

The three DVE ops cost ~1 cycle per output window each (vs ~6 for the
stock tensor_tensor decomposition); ACT runs the sigmoid; the kernel is
then DMA-bound (fp16 in/out, ~21 MB per core).

The REGULAR-mode slot of each op is a poison program (writes -FLT_MAX)
so a silent perf-mode fallback fails the accuracy gate loudly instead
of producing plausible-but-wrong numbers.
"""

import numpy as np

import concourse.bacc as bacc
import concourse.mybir as mybir
import concourse.tile as tile
from concourse import bass_isa
from concourse import dve_ops as _dve_ops_mod
from concourse.bass_utils import run_bass_kernel_spmd
from concourse.dve_uop import (
    AluInp,
    AluOp,
    DelayInp,
    DveOpSpec,
    InpSel,
    OutPath,
    OutSel,
    Trigger,
    UopConfig,
)

F32 = mybir.dt.float32
F16 = mybir.dt.float16

B, H, W, C = 16, 256, 256, 64
N_CORES = 8
BPC = B // N_CORES
HO = H // 2                 # 128 output rows = SBUF partitions
NQ = 4                      # w-quarters per row
WO = W // (2 * NQ)          # output w pairs per macro-tile (32)
FD = WO * C                 # windows per partition per macro-tile (2048)

LAST_EXEC_NS = None
LAST_RESULTS = None

_PROGRAM_CACHE = {}

# --------------------------------------------------------------------------
# Custom DVE ops (hand-authored 2X_1PORT uop programs)
# --------------------------------------------------------------------------

# Block-0 conventions (from the stock TT 2x program): ALU mux PREV_ALU_OUT
# reads input lane 0; mux PREV_DELAY_k reads input lane k+1; delay chain k
# loading DelayInp.PREV_DELAY latches input lane k+1.  At later blocks the
# same selectors read the previous block's flops.
_PD = [
    AluInp.PREV_DELAY_0, AluInp.PREV_DELAY_1, AluInp.PREV_DELAY_2,
    AluInp.PREV_DELAY_3, AluInp.PREV_DELAY_4, AluInp.PREV_DELAY_5,
]


def _exit_on_src_done(u: UopConfig) -> UopConfig:
    u.trigger = (Trigger.SRC_TENSOR_DONE, Trigger.NONE, Trigger.NONE)
    u.next_uop = (0, 0, 0)
    u.require_inp0 = 1
    u.require_inp1 = 1
    return u


def _poison_uop() -> UopConfig:
    """REGULAR-slot program: consume both streams, write -FLT_MAX."""
    u = UopConfig()
    u.enable_input(InpSel.SRC_0, 0)
    u.enable_input(InpSel.SRC_1, 1)
    u.enable_input(InpSel.MAX_NEG, 2)
    u.datapath_config[0].enable_alu(AluOp.BYPASS, _PD[1], _PD[1])
    for k in range(1, 8):
        u.datapath_config[k].pass_through_alu()
    u.enable_output(OutSel.ALU_OUT, OutPath.WR0_LO)
    return _exit_on_src_done(u)


def _pairpool_2x() -> UopConfig:
    """[x1|x2] pairs from (a,b) on port0 and (c,d) on port1.

    x1 = max(a,b,c,d); x2 = K*(a+b+c+d), K = imm0 (0.25)."""
    u = UopConfig()
    u.enable_input(InpSel.SRC_0, 0)      # a      (block0 ALU)
    u.enable_input(InpSel.SRC_1, 1)      # c      -> chain0
    u.enable_input(InpSel.SRC_0_HI, 2)   # b      -> chain1
    u.enable_input(InpSel.SRC_1_HI, 3)   # d      -> chain2
    u.enable_input(InpSel.SRC_0, 4)      # a dup  -> chain3
    u.enable_input(InpSel.CONST_0, 5)    # K      -> chain4
    dp = u.datapath_config
    # b0: p = a + b ; latch c,b,d,a,K into chains 0..4
    dp[0].enable_alu(AluOp.ADD, AluInp.PREV_ALU_OUT, _PD[1])
    for ch in range(5):
        dp[0].enable_delay_from_src(DelayInp.PREV_DELAY, ch)
    # b1: q = c + d ; capture p -> chain5; pass 0..4
    dp[1].enable_alu(AluOp.ADD, _PD[0], _PD[2])
    dp[1].enable_delay_from_src(DelayInp.PREV_ALU_OUT, 5)
    dp[1].pass_through_delay(0, 1, 2, 3, 4)
    # b2: m1 = max(a, b) ; capture q -> chain1; pass 0,2,4,5
    dp[2].enable_alu(AluOp.MAX, _PD[3], _PD[1])
    dp[2].enable_delay_from_src(DelayInp.PREV_ALU_OUT, 1)
    dp[2].pass_through_delay(0, 2, 4, 5)
    # b3: m2 = max(c, d) ; capture m1 -> chain0; pass 1,4,5
    dp[3].enable_alu(AluOp.MAX, _PD[0], _PD[2])
    dp[3].enable_delay_from_src(DelayInp.PREV_ALU_OUT, 0)
    dp[3].pass_through_delay(1, 4, 5)
    # b4: s = p + q ; capture m2 -> chain1; pass 0,4
    dp[4].enable_alu(AluOp.ADD, _PD[5], _PD[1])
    dp[4].enable_delay_from_src(DelayInp.PREV_ALU_OUT, 1)
    dp[4].pass_through_delay(0, 4)
    # b5: x1 = max(m1, m2) ; capture s -> chain0; pass 4
    dp[5].enable_alu(AluOp.MAX, _PD[0], _PD[1])
    dp[5].enable_delay_from_src(DelayInp.PREV_ALU_OUT, 0)
    dp[5].pass_through_delay(4)
    # b6: x2 = s * K ; capture x1 -> chain0
    dp[6].enable_alu(AluOp.MULTIPLY, _PD[0], _PD[4])
    dp[6].enable_delay_from_src(DelayInp.PREV_ALU_OUT, 0)
    # b7: bypass x2 ; pass x1
    dp[7].pass_through_alu()
    dp[7].pass_through_delay(0)
    u.enable_output(OutSel.DELAY_0, OutPath.WR0_LO)   # x1
    u.enable_output(OutSel.ALU_OUT, OutPath.WR0_HI)   # x2
    return _exit_on_src_done(u)


def _pairdot_2x() -> UopConfig:
    """[t3|t3] pairs: t3 = va*a + vb*b + vc*c + d  (va,vb,vc = imm0..2)."""
    u = UopConfig()
    u.enable_input(InpSel.SRC_0, 0)      # a
    u.enable_input(InpSel.SRC_1, 1)      # c   -> chain0
    u.enable_input(InpSel.SRC_0_HI, 2)   # b   -> chain1
    u.enable_input(InpSel.SRC_1_HI, 3)   # d   -> chain2
    u.enable_input(InpSel.CONST_0, 4)    # va  -> chain3
    u.enable_input(InpSel.CONST_1, 5)    # vb  -> chain4
    u.enable_input(InpSel.CONST_2, 6)    # vc  -> chain5
    dp = u.datapath_config
    # b0: A = a * va ; latch c,b,d,vb,vc
    dp[0].enable_alu(AluOp.MULTIPLY, AluInp.PREV_ALU_OUT, _PD[3])
    for ch in (0, 1, 2, 4, 5):
        dp[0].enable_delay_from_src(DelayInp.PREV_DELAY, ch)
    # b1: Bv = b * vb ; capture A -> chain3; pass 0,2,5
    dp[1].enable_alu(AluOp.MULTIPLY, _PD[1], _PD[4])
    dp[1].enable_delay_from_src(DelayInp.PREV_ALU_OUT, 3)
    dp[1].pass_through_delay(0, 2, 5)
    # b2: Cv = c * vc ; capture Bv -> chain4; pass 2,3
    dp[2].enable_alu(AluOp.MULTIPLY, _PD[0], _PD[5])
    dp[2].enable_delay_from_src(DelayInp.PREV_ALU_OUT, 4)
    dp[2].pass_through_delay(2, 3)
    # b3: t = Cv + d ; pass 3,4
    dp[3].enable_alu(AluOp.ADD, AluInp.PREV_ALU_OUT, _PD[2])
    dp[3].pass_through_delay(3, 4)
    # b4: t += A ; pass 4
    dp[4].enable_alu(AluOp.ADD, AluInp.PREV_ALU_OUT, _PD[3])
    dp[4].pass_through_delay(4)
    # b5: t3 = t + Bv
    dp[5].enable_alu(AluOp.ADD, AluInp.PREV_ALU_OUT, _PD[4])
    # b6, b7: bypass
    dp[6].pass_through_alu()
    dp[7].pass_through_alu()
    u.enable_output(OutSel.ALU_OUT, OutPath.WR0_LO)
    u.enable_output(OutSel.ALU_OUT, OutPath.WR0_HI)
    return _exit_on_src_done(u)


def _gate_2x() -> UopConfig:
    """[o|o] pairs: o = x2 + z*(x1 - x2); port0 = [x1|x2], port1 = [z|z].

    (In 2x mode the write side always emits both 16-bit halves, so the
    result is duplicated into a pair-sized dst and compacted on ACT.)"""
    u = UopConfig()
    u.enable_input(InpSel.SRC_0, 0)      # x1
    u.enable_input(InpSel.SRC_1, 1)      # z   -> chain0
    u.enable_input(InpSel.SRC_0_HI, 2)   # x2  -> chain1
    dp = u.datapath_config
    # b0: D = x1 - x2 ; latch z, x2
    dp[0].enable_alu(AluOp.SUBTRACT, AluInp.PREV_ALU_OUT, _PD[1])
    dp[0].enable_delay_from_src(DelayInp.PREV_DELAY, 0)
    dp[0].enable_delay_from_src(DelayInp.PREV_DELAY, 1)
    # b1: G = D * z ; pass x2
    dp[1].enable_alu(AluOp.MULTIPLY, AluInp.PREV_ALU_OUT, _PD[0])
    dp[1].pass_through_delay(1)
    # b2: O = G + x2
    dp[2].enable_alu(AluOp.ADD, AluInp.PREV_ALU_OUT, _PD[1])
    for k in range(3, 8):
        dp[k].pass_through_alu()
    u.enable_output(OutSel.ALU_OUT, OutPath.WR0_LO)
    u.enable_output(OutSel.ALU_OUT, OutPath.WR0_HI)
    return _exit_on_src_done(u)


def _ref_pairpool(in0, in1, c0, c1, c2):
    a = in0[:, 0::2].astype(np.float32)
    b = in0[:, 1::2].astype(np.float32)
    c = in1[:, 0::2].astype(np.float32)
    d = in1[:, 1::2].astype(np.float32)
    out = np.empty(in0.shape, np.float32)
    out[:, 0::2] = np.maximum(np.maximum(a, b), np.maximum(c, d))
    out[:, 1::2] = (a + b + c + d) * np.float32(c0)
    return out


def _ref_pairdot(in0, in1, c0, c1, c2):
    a = in0[:, 0::2].astype(np.float32)
    b = in0[:, 1::2].astype(np.float32)
    c = in1[:, 0::2].astype(np.float32)
    d = in1[:, 1::2].astype(np.float32)
    t3 = c0 * a + c1 * b + c2 * c + d
    out = np.empty(in0.shape, np.float32)
    out[:, 0::2] = t3
    out[:, 1::2] = t3
    return out


def _ref_gate(in0, in1, c0, c1, c2):
    x1 = in0[:, 0::2].astype(np.float32)
    x2 = in0[:, 1::2].astype(np.float32)
    z = in1[:, 0::2].astype(np.float32)
    o = x2 + z * (x1 - x2)
    out = np.empty(in0.shape, np.float32)
    out[:, 0::2] = o
    out[:, 1::2] = o
    return out


class _SpecLike:
    def __init__(self, reference):
        self.reference = reference


class _HandOp:
    """DveOp-alike backed by a hand-written DveOpSpec (2X_1PORT program +
    poison REGULAR slot)."""

    def __init__(self, name, uop_2x, reference):
        self.name = name
        self.spec = _SpecLike(reference)
        self._uop_2x = uop_2x
        self._cache = {}

    def compile(self, ver):
        assert ver == "v3", ver
        if ver not in self._cache:
            s = DveOpSpec(
                name=self.name,
                opcode=_dve_ops_mod.get_dve_sub_opcode(self.name),
                uops=[_poison_uop()],
                uops_2x=[self._uop_2x()],
                perf_max=1,
                rd1_en=True,
            )
            s.validate(ver)
            self._cache[ver] = s
        return self._cache[ver]


# Version-suffixed names: the NEFF cache keys on the BIR json (op names,
# not uop bytes), so any uop-program edit must bump these.
PAIRPOOL = _HandOp("ANT_PAIRPOOL_V1", _pairpool_2x, _ref_pairpool)
PAIRDOT = _HandOp("ANT_PAIRDOT_V1", _pairdot_2x, _ref_pairdot)
GATE = _HandOp("ANT_GATE_V2", _gate_2x, _ref_gate)


def _register_ops():
    by_name = {op.name for op in _dve_ops_mod.OPS}
    for op in (PAIRPOOL, PAIRDOT, GATE):
        if op.name in by_name:
            continue
        row = _dve_ops_mod._CUSTOM_DVE_ROW_BASE + len(_dve_ops_mod.OPS)
        assert row < 0x20, row
        _dve_ops_mod.OPS.append(op)
        _dve_ops_mod._SUB_OPCODE_FOR_NAME[op.name] = row
        _dve_ops_mod.CUSTOM_DVE_SPECS[op.name] = op.spec


_register_ops()


def _emit_custom(nc, op, out, in0, in1, s0=0.0, s1=0.0, imm2=0.0):
    """Emit one custom-DVE instruction (mirrors bass._custom_dve for the
    TTSS shape with float scalars)."""
    eng = nc.vector
    if op.name not in nc.m.ant_custom_dve_ops:
        nc.m.ant_custom_dve_ops = sorted({*nc.m.ant_custom_dve_ops, op.name})
    isa_opcode = nc.isa.Opcode[
        f"NEURON_ISA_TPB_OPCODE_CUSTOM_DVE_ANT_{bass_isa.CustomDveShape.TTSS.slot()}"
    ].value
    ins = [
        eng.lower_ap(in0, for_isa=True),
        eng.lower_ap(in1, for_isa=True),
        mybir.ImmediateValue(dtype=F32, value=float(s0)),
        mybir.ImmediateValue(dtype=F32, value=float(s1)),
    ]
    outs = [eng.lower_ap(out, for_isa=True)]
    return eng.add_instruction(
        mybir.InstCustomDveAnt(
            name=nc.get_next_instruction_name(),
            op_name=op.name,
            rd1_en=True,
            subdim=0,
            imm2=float(imm2),
            shape=bass_isa.CustomDveShape.TTSS,
            row=_dve_ops_mod.get_dve_sub_opcode(op.name),
            isa_opcode=isa_opcode,
            ins=ins,
            outs=outs,
            perf_max=1,
        )
    )


# --------------------------------------------------------------------------
# Kernel program
# --------------------------------------------------------------------------


def _build_program(bpc, nq, wo, ch, coef):
    """Single-core Bass/Tile program.  coef = (va, vb, vc, f): the host
    permutes window corners so the largest-|mask| corner is slot d; then
    xs = f * (va*a + vb*b + vc*c + d) with |v*| <= 1."""
    from contextlib import ExitStack

    va, vb, vc, f = coef
    fd = wo * ch               # windows per partition per tile

    nc = bacc.Bacc(
        "TRN2",
        target_bir_lowering=False,
        debug=False,
        enable_asserts=True,
        num_devices=N_CORES,
    )

    # x layout per tile: [E-plane | O-plane], each plane fd pairs (w', c, e)
    x = nc.dram_tensor("x", [bpc, HO, nq, 4 * fd], F16, kind="ExternalInput")
    out = nc.dram_tensor("out", [bpc, HO, nq, fd], F16, kind="ExternalOutput")
    x_ap = x.ap()
    out_ap = out.ap()

    with tile.TileContext(nc) as tc, ExitStack() as ctx:
        pool_io = ctx.enter_context(tc.tile_pool(name="io", bufs=7))
        pool_pp = ctx.enter_context(tc.tile_pool(name="pp", bufs=2))
        pool_t3 = ctx.enter_context(tc.tile_pool(name="t3", bufs=2))
        pool_z = ctx.enter_context(tc.tile_pool(name="z", bufs=1))
        pool_od = ctx.enter_context(tc.tile_pool(name="od", bufs=2))
        pool_out = ctx.enter_context(tc.tile_pool(name="outp", bufs=2))

        z_buf0 = pool_z.tile([128, 2 * FD], F16, tag="z0")
        z_buf1 = pool_z.tile([128, 2 * FD], F16, tag="z1")
        z_bufs = [z_buf0, z_buf1]
        for zb in z_bufs:
            nc.vector.memset(zb[:], 0.0)

        def emit_load(b, q, w_lo, w_hi, issuer=None):
            nw = w_hi - w_lo
            fdw = nw * ch
            EO_t = pool_io.tile([128, 4 * fd], F16, tag="EO")
            EO = EO_t[:, 0 : 4 * fdw]
            # src: [r(2), w', c, e] with w' sliced
            src = x_ap[b, :, q, :].rearrange(
                "p (r w ce) -> p r w ce", r=2, ce=2 * ch
            )[:, :, w_lo:w_hi, :]
            (issuer or nc.sync).dma_start(
                EO.rearrange("p (r w ce) -> p r w ce", r=2, ce=2 * ch), src
            )
            return dict(b=b, q=q, w_lo=w_lo, nw=nw, EO=EO)

        def emit_front(h):
            nw = h["nw"]
            fdw = nw * ch
            EO = h["EO"]
            E = EO[:, 0 : 2 * fdw]
            O = EO[:, 2 * fdw : 4 * fdw]
            PP_t = pool_pp.tile([128, 2 * fd], F16, tag="PP")
            PP = PP_t[:, 0 : 2 * fdw]
            _emit_custom(nc, PAIRPOOL, PP, E, O, s0=0.25)
            T3_t = pool_t3.tile([128, 2 * fd], F16, tag="T3")
            T3 = T3_t[:, 0 : 2 * fdw]
            _emit_custom(nc, PAIRDOT, T3, E, O, s0=va, s1=vb, imm2=vc)
            Z = z_bufs[h["idx"] % 2][:, 0 : 2 * fdw]
            t3_lo = T3.rearrange("p (w two) -> p w two", two=2)[:, :, 0]
            z_lo = Z.rearrange("p (w two) -> p w two", two=2)[:, :, 0]
            nc.scalar.activation(
                z_lo, t3_lo, mybir.ActivationFunctionType.Sigmoid,
                bias=0.0, scale=float(f),
            )
            h.update(PP=PP, Z=Z)



        def emit_back(h):
            b, q, w_lo, nw = h["b"], h["q"], h["w_lo"], h["nw"]
            fdw = nw * ch
            od_t = pool_od.tile([128, 2 * fd], F16, tag="od")
            od = od_t[:, 0 : 2 * fdw]
            _emit_custom(nc, GATE, od, h["PP"], h["Z"])
            o_t = pool_out.tile([128, fd], F16, tag="o")
            o = o_t[:, 0:fdw]
            od_lo = od.rearrange("p (w two) -> p w two", two=2)[:, :, 0]
            nc.scalar.copy(o, od_lo)
            dst = out_ap[b, :, q, :].rearrange("p (w c) -> p w c", c=ch)
            nc.sync.dma_start(
                dst[:, w_lo : w_lo + nw, :],
                o.rearrange("p (w c) -> p w c", c=ch),
            )

        tiles = []
        n_mt = bpc * nq
        for b in range(bpc):
            for q in range(nq):
                mt = b * nq + q
                if mt == 0:
                    # quarter the first macro-tile: fast pipeline ramp
                    for k in range(4):
                        tiles.append((b, q, k * wo // 4, (k + 1) * wo // 4))
                elif mt == 1:
                    tiles.append((b, q, 0, wo // 2))
                    tiles.append((b, q, wo // 2, wo))
                elif mt == n_mt - 1:
                    # medium tail tiles: short final chain but still long
                    # enough to hide the sigmoid latency between rounds
                    tiles.append((b, q, 0, wo // 2))
                    tiles.append((b, q, wo // 2, wo))
                else:
                    tiles.append((b, q, 0, wo))

        pf = min(5, len(tiles))
        hs = []
        for j in range(pf):
            hh = emit_load(*tiles[j])
            hh["idx"] = j
            hs.append(hh)
        _next_idx = [pf]
        emit_front(hs[0])
        for i in range(1, len(tiles)):
            if i + pf - 1 < len(tiles):
                hh = emit_load(*tiles[i + pf - 1])
                hh["idx"] = _next_idx[0]
                _next_idx[0] += 1
                hs.append(hh)
            emit_front(hs[i])
            emit_back(hs[i - 1])
        emit_back(hs[-1])

    nc.compile()
    return nc


def _get_program(bpc, nq, wo, ch, coef):
    key = (bpc, nq, wo, ch, coef)
    if key not in _PROGRAM_CACHE:
        _PROGRAM_CACHE[key] = _build_program(bpc, nq, wo, ch, coef)
    return _PROGRAM_CACHE[key]


def _mask_coef(mask):
    """Choose corner flips so the largest-|m| corner sits at slot d, and
    return ((flip_r, flip_e), (va, vb, vc, f))."""
    m = np.asarray(mask, np.float64)
    ri, ei = np.unravel_index(np.argmax(np.abs(m)), (2, 2))
    fr, fe = int(ri ^ 1), int(ei ^ 1)
    # corner at packed position (pr, pe) is m[pr ^ fr, pe ^ fe]
    f = m[1 ^ fr, 1 ^ fe]
    if f == 0.0:
        # all-zero mask: z == 0.5 everywhere; keep ratios 0
        return (fr, fe), (0.0, 0.0, 0.0, 0.0)
    va = m[0 ^ fr, 0 ^ fe] / f
    vb = m[0 ^ fr, 1 ^ fe] / f
    vc = m[1 ^ fr, 0 ^ fe] / f
    return (fr, fe), (float(va), float(vb), float(vc), float(f))


def _host_pack(x, fr, fe, bs=None, nq=NQ, wo=WO):
    """[B', H, W, C] f32 -> fp16 [B', HO, nq, (r w' c e)] pair layout."""
    bs = x.shape[0] if bs is None else bs
    xh = np.ascontiguousarray(x, np.float32).astype(np.float16)
    xh = xh.reshape(bs, HO, 2, nq, wo, 2, C)       # b h' r q w' e c
    if fr:
        xh = xh[:, :, ::-1]
    if fe:
        xh = xh[:, :, :, :, :, ::-1]
    xh = xh.transpose(0, 1, 3, 2, 4, 6, 5)         # b h' q r w' c e
    return np.ascontiguousarray(xh).reshape(bs, HO, nq, 4 * wo * C)


def kernel(x, mask):
    import os

    global LAST_EXEC_NS, LAST_RESULTS

    x = np.asarray(x)
    mask = np.asarray(mask)
    assert x.shape == (B, H, W, C), x.shape
    in_dtype = x.dtype

    (fr, fe), coef = _mask_coef(mask)
    nc = _get_program(BPC, NQ, WO, C, coef)

    xv = _host_pack(x, fr, fe)

    in_maps = [{"x": xv[i * BPC : (i + 1) * BPC]} for i in range(N_CORES)]

    trace = os.environ.get("KERNEL_TRACE", "0") == "1"
    res = run_bass_kernel_spmd(
        nc, in_maps, core_ids=list(range(N_CORES)), trace=trace
    )
    LAST_EXEC_NS = res.exec_time_ns
    LAST_RESULTS = res

    parts = [r["out"].reshape(BPC, HO, NQ * WO, C) for r in res.results]
    full = np.concatenate(parts, axis=0)
    return full.astype(in_dtype, copy=False)


def _numpy_reference(x, mask):
    xr = x.reshape(x.shape[0], x.shape[1] // 2, 2, x.shape[2] // 2, 2, x.shape[3])
    x1 = xr.max(axis=(2, 4))
    x2 = xr.mean(axis=(2, 4))
    xs = np.einsum("bhiwjc,ij->bhwc", xr, mask)
    z = 1.0 / (1.0 + np.exp(-xs))
    return z * x1 + (1.0 - z) * x2


if __name__ == "__main__":
    from concourse.bass_interp import CoreSim

    rng = np.random.default_rng(0)
    bpc_s, nq_s, wo_s = 1, 2, 8
    h_s, w_s = 256, nq_s * 2 * wo_s
    xs_np = rng.standard_normal((bpc_s, h_s, w_s, C)).astype(np.float32)
    mask_np = (rng.standard_normal((2, 2)) * 0.5).astype(np.float32)

    (fr_s, fe_s), coef_s = _mask_coef(mask_np)
    nc = _build_program(bpc_s, nq_s, wo_s, C, coef_s)
    sim = CoreSim(nc, trace=False)
    sim.tensor("x")[:] = _host_pack(xs_np, fr_s, fe_s, bpc_s, nq_s, wo_s)
    sim.simulate()
    got = (
        sim.tensor("out").reshape(bpc_s, HO, nq_s * wo_s, C).astype(np.float64)
    )
    want = _numpy_reference(xs_np.astype(np.float64), mask_np.astype(np.float64))
    err = np.abs(got - want)
    rel = err.max() / np.abs(want).max()
    print("CoreSim selftest: max abs err", err.max(), "rel", rel)
    assert rel < 3e-3, rel
    print("PASS")
